# revision 8
# baseline (speedup 1.0000x reference)
"""Trainium2 Bass kernel for nn_Encoder_45475113730366 (v2).

Data-parallel over batch (64 -> 8 cores x 8 items). Per item the 4-layer
encoder stack runs on 5 streams (m1, m2, e1, e2, enc).

v2 design (vs v1 baseline):
  - fp16 everywhere on device (weights, activations, states, DRAM IO);
    PSUM accumulation stays fp32.  Host pre-transposes the input to
    feature-major [item, 256, 2048] fp16 and post-transposes the outputs,
    so there are no on-device layout transposes for IO.
  - q-projection eliminated: scores S = wq^T P - cq (x) s with
    P = x^T (rstd*k), s = colmean(P).  LN1 mean/rstd fold algebraically.
  - softmax fused to 7 ops via 3D-AP reduces + stride-0 broadcast TT.
  - attention applied as one [64->128] projection C = bda^T wfcP; the
    block-diagonal attn weights come from masking scores with -1e9.
  - FFN bias b1 folded into h_ps via rank-1 matmuls (b1_j (x) ones).
  - Mish tail = ONE custom DVE op (seed+Chebyshev-NR reciprocal fused
    with the final multiply), after ACT Exp + ACT Square.
"""
from contextlib import ExitStack

import numpy as np

import concourse.bacc as bacc
import concourse.bass as bass
import concourse.tile as tile
from concourse import mybir
from concourse.masks import make_identity

N_CORES = 8
B, S, DM, H, DK, DI, L = 64, 2048, 128, 8, 16, 512, 4
DKP = DK // 2
HE = H * DKP          # 64 pooled kv features
NT = S // 128         # 16 token tiles
EPS = 1e-6
TEMP = 0.5 * float(np.sqrt(DK))
QK = 0x5f3759df       # quake rsqrt seed constant
MISH_EXACT = True

f32 = mybir.dt.float32
f16 = mybir.dt.float16
i32 = mybir.dt.int32
AX = mybir.AxisListType.X
OP = mybir.AluOpType
AF = mybir.ActivationFunctionType

# ---------------------------------------------------------------------------
# custom fused DVE op: mish tail
#   out = Src1 * (1 - y1), y1 ~= 2/(Src0+1) via ~bits seed + Chebyshev-NR.
#   Src0 = w = (1+e^hb)^2 (fp32), Src1 = hb (fp32 PSUM, b1 included).
MC1 = -0.33699572
MC2 = 2.83013085


def _ref_misht(in0, in1, s0, s1, imm2):
    den = (in0.astype(np.float32) + 1.0).astype(np.float32)
    nx = (~den.view(np.uint32)).view(np.float32)
    y0 = nx * np.float32(s1)
    y1 = y0 * (np.float32(imm2) - den * y0)
    return (in1.astype(np.float32) * (1.0 - y1)).astype(np.float32)


def _register_misht():
    import concourse.dve_ops as dv
    from concourse.dve_spec import Spec, Src0, Src1, C1, C2, One, lower
    from concourse.dve_ops import DveOp, Bin
    from concourse.dve_uop import AluOp, DveOpSpec

    if "MISH_T_ANT" in dv._SUB_OPCODE_FOR_NAME:
        return next(o for o in dv.OPS if o.name == "MISH_T_ANT")
    den = Src0 + One
    nx = Bin(AluOp.BITWISE_NOT, den, den)
    y0 = nx * C1
    y1 = y0 * (C2 - den * y0)
    op = DveOp("MISH_T_ANT", Spec(body=Src1 * (One - y1), reference=_ref_misht),
               subdim=False, uops_sha={})
    opc = max(dv._SUB_OPCODE_FOR_NAME.values()) + 1
    assert opc < 0x20
    for ver in ("v3", "v4"):
        try:
            uops = lower(op.spec, ver=ver)
        except Exception:
            continue
        op.uops_sha[ver] = DveOpSpec(name=op.name, opcode=opc, uops=uops).sha(ver)
    dv.OPS.append(op)
    dv.CUSTOM_DVE_SPECS[op.name] = op.spec
    dv._SUB_OPCODE_FOR_NAME[op.name] = opc
    return op


def _emit_misht(nc, out, h_ps, w_sb):
    op = _register_misht()
    return nc.vector._custom_dve(op, out=out, in0=w_sb, in1=h_ps,
                                 s0=0.0, s1=MC1, imm2=MC2)


# ---------------------------------------------------------------------------
def fold_weights(inp):
    f = {}
    Wq = np.asarray(inp['Wq'], np.float32)
    Wk = np.asarray(inp['Wk'], np.float32)
    Wv = np.asarray(inp['Wv'], np.float32)
    Wfc = np.asarray(inp['Wfc'], np.float32)
    W1 = np.asarray(inp['W1'], np.float32)
    W2 = np.asarray(inp['W2'], np.float32)
    g1 = np.asarray(inp['ln1_g'], np.float32)
    b1n = np.asarray(inp['ln1_b'], np.float32)
    g2 = np.asarray(inp['ln2_g'], np.float32)
    b2n = np.asarray(inp['ln2_b'], np.float32)

    wq = ((g1[:, :, None] * Wq) / TEMP).astype(np.float16)       # [L,128,128]
    f['wq'] = wq
    # rank-1 mean-correction row: ncq = -colsum(wq) (fp16-consistent)
    f['ncq'] = (-wq.astype(np.float32).sum(axis=1)).astype(np.float16) \
        .reshape(L, DM)                                          # [L,128]
    bq = np.einsum('ld,ldf->lf', b1n, Wq) / TEMP
    f['bq_trivial'] = bool(np.abs(bq).max() == 0.0)
    f['bqr'] = bq.astype(np.float16)                             # [L,128]
    f['wk'] = Wk.reshape(L, DM, H, DKP, 2).mean(-1).reshape(L, DM, HE) \
        .astype(np.float16)
    f['wv'] = Wv.reshape(L, DM, H, DKP, 2).mean(-1).reshape(L, DM, HE) \
        .astype(np.float16)
    perm = np.array([d * H + h for h in range(H) for d in range(DK)])
    f['wfc'] = Wfc[:, perm, :].astype(np.float16)                # [L,128,128]
    f['w1'] = (g2[:, :, None] * W1).astype(np.float16)           # [L,128,512]
    b1f = np.einsum('ld,ldf->lf', b2n, W1) + np.asarray(inp['b1'], np.float32)
    # rank-1 bias rows, flattened [1, L*4*128]
    f['b1r'] = b1f.reshape(1, L * DI).astype(np.float16)
    # W2 rearranged: chunk j is a [128,128] lhsT at cols j*128:(j+1)*128
    f['w2r'] = W2.reshape(L, 4, 128, DM).transpose(0, 2, 1, 3) \
        .reshape(L, 128, 4 * DM).astype(np.float16)
    f['b2s'] = np.ascontiguousarray(
        np.asarray(inp['b2'], np.float32).T)                     # [128,L]
    f['wl2'] = np.asarray(inp['WL2'], np.float32).astype(np.float16)
    f['bl2'] = np.asarray(inp['bL2'], np.float32).reshape(DM, 1)  # [128,1]

    mask = np.asarray(inp['src_mask'])                           # [B,16,8]
    f['mask_trivial'] = bool(mask.all())
    # score-space mask bias [*, 128, 64]: row p=(h,d), col c=(h',e):
    # 0 where h'==h and mask[d,e], else -1e9
    blk = np.full((DM, HE), -1e9, np.float32)
    mb_all = np.broadcast_to(blk, (mask.shape[0], DM, HE)).copy()
    for h in range(H):
        # on-block region rows h*16:(h+1)*16, cols h*8:(h+1)*8
        sub = mb_all[:, h * DK:(h + 1) * DK, h * DKP:(h + 1) * DKP]
        sub[:] = np.where(mask, 0.0, -1e9)[:, :, :]
    f['mb'] = mb_all                                             # [B,128,64]
    f['mb0'] = mb_all[0]                                         # [128,64]
    return f


# ---------------------------------------------------------------------------
def build(n_items, use_bq, use_mask):
    nc = bacc.Bacc(trn_type="TRN2", target_bir_lowering=False, debug=False)
    _register_misht()

    xin = nc.dram_tensor("xin", [n_items, 2 * DM, S], f16,
                         kind="ExternalInput").ap()
    wq_d = nc.dram_tensor("wq", [L, DM, DM], f16, kind="ExternalInput").ap()
    ncq_d = nc.dram_tensor("ncq", [1, L * DM], f16, kind="ExternalInput").ap()
    wk_d = nc.dram_tensor("wk", [L, DM, HE], f16, kind="ExternalInput").ap()
    wv_d = nc.dram_tensor("wv", [L, DM, HE], f16, kind="ExternalInput").ap()
    wfc_d = nc.dram_tensor("wfc", [L, DM, DM], f16, kind="ExternalInput").ap()
    w1_d = nc.dram_tensor("w1", [L, DM, DI], f16, kind="ExternalInput").ap()
    b1r_d = nc.dram_tensor("b1r", [1, L * DI], f16, kind="ExternalInput").ap()
    w2_d = nc.dram_tensor("w2r", [L, DM, DI], f16, kind="ExternalInput").ap()
    b2_d = nc.dram_tensor("b2s", [DM, L], f32, kind="ExternalInput").ap()
    wl2_d = nc.dram_tensor("wl2", [2 * DM, DM], f16, kind="ExternalInput").ap()
    bl2_d = nc.dram_tensor("bl2", [DM, 1], f32, kind="ExternalInput").ap()
    crow_d = nc.dram_tensor("crow", [1, DI], f16, kind="ExternalInput").ap()
    if use_mask:
        mb_d = nc.dram_tensor("mb", [n_items, DM, HE], f32,
                              kind="ExternalInput").ap()
    else:
        mb_d = nc.dram_tensor("mb0", [DM, HE], f32, kind="ExternalInput").ap()
    if use_bq:
        bqr_d = nc.dram_tensor("bqr", [1, L * DM], f16,
                               kind="ExternalInput").ap()
    m1_o = nc.dram_tensor("m1o", [n_items, DM, S], f16,
                          kind="ExternalOutput").ap()
    m2_o = nc.dram_tensor("m2o", [n_items, DM, S], f16,
                          kind="ExternalOutput").ap()
    e_o = nc.dram_tensor("eo", [n_items, DM, S], f16,
                         kind="ExternalOutput").ap()

    with tile.TileContext(nc) as tc, ExitStack() as ctx:
        consts = ctx.enter_context(tc.tile_pool(name="consts", bufs=1))
        bigp = ctx.enter_context(tc.tile_pool(name="bigp", bufs=1))
        statep = ctx.enter_context(tc.tile_pool(name="statep", bufs=4))
        workp = ctx.enter_context(tc.tile_pool(name="workp", bufs=2))
        tmpp = ctx.enter_context(tc.tile_pool(name="tmpp", bufs=2))
        tinyp = ctx.enter_context(tc.tile_pool(name="tinyp", bufs=3))
        rowp = ctx.enter_context(tc.tile_pool(name="rowp", bufs=2))
        ps_tiny = ctx.enter_context(
            tc.tile_pool(name="ps_tiny", bufs=2, space="PSUM"))
        ps_big = ctx.enter_context(
            tc.tile_pool(name="ps_big", bufs=2, space="PSUM"))
        ps_o = ctx.enter_context(
            tc.tile_pool(name="ps_o", bufs=1, space="PSUM"))

        # ---- constants / weights ------------------------------------
        identf = consts.tile([128, 128], f32, tag="identf")
        make_identity(nc, identf)
        ident16 = consts.tile([128, 128], f16, tag="ident16")
        nc.vector.tensor_copy(ident16, identf)
        ones128 = consts.tile([128, 1], f16, tag="ones128")
        nc.vector.memset(ones128, 1.0 / 128.0)
        onesrow = consts.tile([1, DI], f16, tag="onesrow")
        nc.sync.dma_start(out=onesrow, in_=crow_d)

        def _load(name, dram_ap, shape, dt=f16):
            t = consts.tile(list(shape), dt, tag=name)
            nc.sync.dma_start(out=t, in_=dram_ap)
            return t

        wq_sb = [_load(f"wq{i}", wq_d[i], [128, DM]) for i in range(L)]
        wk_sb = [_load(f"wk{i}", wk_d[i], [128, HE]) for i in range(L)]
        wv_sb = [_load(f"wv{i}", wv_d[i], [128, HE]) for i in range(L)]
        wfc_sb = [_load(f"wfc{i}", wfc_d[i], [128, DM]) for i in range(L)]
        w1_sb = [_load(f"w1{i}", w1_d[i], [128, DI]) for i in range(L)]
        w2_sb = [_load(f"w2{i}", w2_d[i], [128, DI]) for i in range(L)]
        ncq_sb = _load("ncq", ncq_d, [1, L * DM])
        b1r_sb = _load("b1r", b1r_d, [1, L * DI])
        b2_sb = _load("b2s", b2_d, [DM, L], f32)
        wl2a = _load("wl2a", wl2_d[0:DM], [128, DM])
        wl2b = _load("wl2b", wl2_d[DM:2 * DM], [128, DM])
        bl2_sb = _load("bl2", bl2_d, [128, 1], f32)
        if use_bq:
            bqr_sb = _load("bqr", bqr_d, [1, L * DM])
        if not use_mask:
            mb0_sb = _load("mb0", mb_d, [DM, HE], f32)

        def ap3(t, d1, d2, psrc=None):
            """view SBUF/PSUM 2D tile [128, d1*d2] as [128, d1, d2]"""
            src = psrc if psrc is not None else t
            return bass.AP(tensor=src.tensor, offset=src.offset,
                           ap=[src.ap[0], [d2, d1], [1, d2]])

        def bcast3(t, d1, d2):
            """[128, d1] -> [128, d1, d2] stride-0 broadcast"""
            return bass.AP(tensor=t.tensor, offset=t.offset,
                           ap=[t.ap[0], [1, d1], [0, d2]])

        # ---- per-layer emission --------------------------------------
        def rsqrt_neg(v):
            """[-rstd] via quake seed + 3 Newton iters, [128,16] f32."""
            yi = tinyp.tile([128, 16], i32, tag="yi")
            nc.vector.tensor_scalar(out=yi, in0=v.bitcast(i32), scalar1=1,
                                    scalar2=None, op0=OP.arith_shift_right)
            nc.vector.tensor_scalar(out=yi, in0=yi, scalar1=-1,
                                    scalar2=None, op0=OP.bitwise_xor)
            nc.vector.tensor_scalar(out=yi, in0=yi, scalar1=QK + 1,
                                    scalar2=None, op0=OP.add)
            y = yi.bitcast(f32)
            hv = tinyp.tile([128, 16], f32, tag="hv")
            nc.vector.tensor_scalar(out=hv, in0=v, scalar1=0.5, scalar2=None,
                                    op0=OP.mult)
            tq = tinyp.tile([128, 16], f32, tag="tq")
            for _ in range(3):
                nc.vector.tensor_mul(tq, y, y)
                nc.vector.tensor_mul(tq, tq, hv)
                nc.vector.scalar_tensor_tensor(out=y, in0=tq, scalar=1.5, in1=y,
                                               op0=OP.subtract, op1=OP.mult)
            return y

        def stats(xtok):
            """token-major xtok [128,2048] -> (mu, e2) [128,16] f32 SBUF."""
            mu = tinyp.tile([128, 16], f32, tag="mu")
            nc.vector.tensor_reduce(out=mu, in_=ap3(xtok, NT, 128), axis=AX,
                                    op=OP.add)
            nc.vector.tensor_scalar(out=mu, in0=mu, scalar1=1.0 / 128.0,
                                    scalar2=None, op0=OP.mult)
            sq = tmpp.tile([128, S], f16, tag="sqt")
            for c in range(4):
                nc.gpsimd.tensor_mul(sq[:, c * 512:(c + 1) * 512],
                                     xtok[:, c * 512:(c + 1) * 512],
                                     xtok[:, c * 512:(c + 1) * 512])
            e2 = tinyp.tile([128, 16], f32, tag="e2")
            nc.vector.tensor_reduce(out=e2, in_=ap3(sq, NT, 128), axis=AX,
                                    op=OP.add)
            return mu, e2

        def chain_nrstd(mu, e2):
            """(mu, e2) -> -rstd [128,16] f32 (e2 is a raw sum of squares)."""
            musq = tinyp.tile([128, 16], f32, tag="musq")
            nc.vector.tensor_mul(musq, mu, mu)
            vpe = tinyp.tile([128, 16], f32, tag="vpe")
            nc.vector.scalar_tensor_tensor(out=vpe, in0=e2,
                                           scalar=1.0 / 128.0,
                                           in1=musq, op0=OP.mult,
                                           op1=OP.subtract)
            nc.vector.tensor_scalar(out=vpe, in0=vpe, scalar1=float(EPS),
                                    scalar2=None, op0=OP.add)
            return rsqrt_neg(vpe)

        def tokmajor(x, eng=None):
            """feature-major [128,S] -> per-tile token-major via one DMA.
            out[p, t*128+f] = x[f, t*128+p]."""
            xt = workp.tile([128, NT, 128], f16, tag="xtok")
            (eng or nc.sync).dma_start_transpose(out=xt, in_=x)
            return xt.rearrange("p a b -> p (a b)")

        def emit_layer(i, xq, xkv, mb_sb):
            # ---- token-major xq + LN1 stats + k projection ------------
            xqtok = tokmajor(xq)
            mu1, e21 = stats(xqtok)
            k_ps = ps_big.tile([128, 1024], f32, tag="big")
            for t in range(NT):
                nc.tensor.matmul(k_ps[:, t * HE:(t + 1) * HE],
                                 lhsT=xkv[:, t * 128:(t + 1) * 128],
                                 rhs=wk_sb[i])
            ty = ps_tiny.tile([128, 512], f32, tag="ty")
            nrstd = chain_nrstd(mu1, e21)
            rstd = tinyp.tile([128, 16], f32, tag="rstd")
            nc.vector.tensor_scalar(out=rstd, in0=nrstd, scalar1=-1.0,
                                    scalar2=None, op0=OP.mult)

            # ---- k scale (ACT, per token tile) ------------------------
            k_sb = workp.tile([128, NT * HE], f16, tag="ksb")
            for t in range(NT):
                nc.scalar.activation(k_sb[:, t * HE:(t + 1) * HE],
                                     k_ps[:, t * HE:(t + 1) * HE],
                                     AF.Identity, scale=rstd[:, t:t + 1])

            # ---- P accumulation + scores ------------------------------
            p_ps = ty[:, 0:HE]
            for t in range(NT):
                nc.tensor.matmul(p_ps,
                                 lhsT=xqtok[:, t * 128:(t + 1) * 128],
                                 rhs=k_sb[:, t * HE:(t + 1) * HE],
                                 start=(t == 0), stop=(t == NT - 1))
            p_sb = tinyp.tile([128, HE], f16, tag="psb")
            nc.vector.tensor_copy(p_sb, p_ps)
            s_ps = ty[0:1, HE:2 * HE]
            nc.tensor.matmul(s_ps, lhsT=ones128, rhs=p_sb)
            s_sb = tinyp.tile([1, HE], f16, tag="ssb")
            nc.vector.tensor_copy(s_sb, s_ps)
            sc_ps = ty[:, 2 * HE:3 * HE]
            nc.tensor.matmul(sc_ps, lhsT=wq_sb[i], rhs=p_sb,
                             start=True, stop=False)
            ncq_row = ncq_sb[:, i * DM:(i + 1) * DM]
            nc.tensor.matmul(sc_ps, lhsT=ncq_row, rhs=s_sb,
                             start=False, stop=(not use_bq))
            if use_bq:
                xsum = tinyp.tile([128, 1], f32, tag="xsum")
                nc.vector.tensor_reduce(out=xsum, in_=ap3(xkv, 4, 512),
                                        axis=mybir.AxisListType.XY, op=OP.add)
                xsum16 = tinyp.tile([128, 1], f16, tag="xsum16")
                nc.vector.tensor_copy(xsum16, xsum)
                ks_ps = ty[0:1, 3 * HE:4 * HE]
                nc.tensor.matmul(ks_ps, lhsT=xsum16, rhs=wk_sb[i])
                ks_sb = tinyp.tile([1, HE], f16, tag="kss")
                nc.vector.tensor_copy(ks_sb, ks_ps)
                nc.tensor.matmul(sc_ps, lhsT=bqr_sb[:, i * DM:(i + 1) * DM],
                                 rhs=ks_sb, start=False, stop=True)

            # ---- softmax -> block-diagonal attn (fp16) ----------------
            sm = tinyp.tile([128, HE], f32, tag="sm")
            nc.vector.tensor_tensor(out=sm, in0=sc_ps, in1=mb_sb, op=OP.add)
            negmx = tinyp.tile([128, 1], f32, tag="negmx")
            nc.vector.tensor_reduce(out=negmx, in_=sm, axis=AX,
                                    op=OP.max, negate=True)
            sm2 = tinyp.tile([128, HE], f32, tag="sm2")
            nc.vector.tensor_scalar(out=sm2, in0=sm, scalar1=negmx,
                                    scalar2=None, op0=OP.add)
            es = tinyp.tile([128, HE], f32, tag="es")
            nc.scalar.activation(es, sm2, AF.Exp)
            ssum = tinyp.tile([128, H], f32, tag="ssum")
            nc.vector.tensor_reduce(out=ssum, in_=ap3(es, H, DKP), axis=AX,
                                    op=OP.add)
            nc.vector.tensor_scalar(out=ssum, in0=ssum, scalar1=1e-30,
                                    scalar2=None, op0=OP.add)
            rs = tinyp.tile([128, H], f32, tag="rs")
            nc.vector.reciprocal(rs, ssum)
            bda = tinyp.tile([128, HE], f16, tag="bda")
            nc.vector.tensor_tensor(out=ap3(bda, H, DKP), in0=ap3(es, H, DKP),
                                    in1=bcast3(rs, H, DKP), op=OP.mult)

            # ---- C = bda^T wfcP ---------------------------------------
            c_ps = ty[0:HE, 4 * HE:4 * HE + 128]
            nc.tensor.matmul(c_ps, lhsT=bda, rhs=wfc_sb[i])
            c_sb = tinyp.tile([HE, 128], f16, tag="csb")
            nc.vector.tensor_copy(c_sb, c_ps)

            # ---- v projection + attn out + residual -------------------
            vT = workp.tile([HE, S], f16, tag="vt")
            for c in range(4):
                v_ps = ps_big.tile([HE, 512], f32, tag="big")
                nc.tensor.matmul(v_ps, lhsT=wv_sb[i],
                                 rhs=xkv[:, c * 512:(c + 1) * 512])
                nc.scalar.copy(out=vT[:, c * 512:(c + 1) * 512], in_=v_ps)
            out1 = workp.tile([128, S], f16, tag="o1")
            for c in range(4):
                cs = slice(c * 512, (c + 1) * 512)
                ao_ps = ps_big.tile([128, 512], f32, tag="big")
                nc.tensor.matmul(ao_ps, lhsT=c_sb, rhs=vT[:, cs])
                nc.vector.tensor_tensor(out=out1[:, cs], in0=ao_ps,
                                        in1=xq[:, cs], op=OP.add)

            # ---- LN2 stats + rows -------------------------------------
            o1tok = tokmajor(out1)
            mu2, e22 = stats(o1tok)
            nrstd2 = chain_nrstd(mu2, e22)
            r2h = tinyp.tile([128, 16], f16, tag="r2h")
            nc.vector.tensor_scalar(out=r2h, in0=nrstd2, scalar1=-1.0,
                                    scalar2=None, op0=OP.mult)
            nm2h = tinyp.tile([128, 16], f16, tag="nm2h")
            nc.vector.tensor_tensor(out=nm2h, in0=mu2, in1=nrstd2, op=OP.mult)
            tr_ps = ty[0:16, 384:512].bitcast(f16)
            nc.tensor.transpose(tr_ps[:, 0:128], r2h, ident16)
            nc.tensor.transpose(tr_ps[:, 128:256], nm2h, ident16)
            rows = rowp.tile([16, 256], f16, tag="rows")
            nc.vector.tensor_copy(rows, tr_ps)
            rowrow = rowp.tile([1, 2 * S], f16, tag="rr")
            r2row = rowrow[:, 0:S]
            nmrow = rowrow[:, S:2 * S]
            nc.sync.dma_start(out=r2row, in_=rows[:, 0:128])
            nc.sync.dma_start(out=nmrow, in_=rows[:, 128:256])

            # ---- FFN per 1024-token chunk -----------------------------
            out2 = statep.tile([128, S], f16, tag="state")
            for c2 in range(2):
                cs2 = slice(c2 * 1024, (c2 + 1) * 1024)
                n2c = tmpp.tile([128, 1024], f16, tag="n2c")
                for cc in range(2):
                    c = 2 * c2 + cc
                    s5 = slice(c * 512, (c + 1) * 512)
                    l5 = slice(cc * 512, (cc + 1) * 512)
                    bc_ps = ps_big.tile([128, 1024], f32, tag="big")
                    nc.tensor.matmul(bc_ps[:, 0:512],
                                     lhsT=onesrow[:, 0:128], rhs=r2row[:, s5])
                    nc.tensor.matmul(bc_ps[:, 512:1024],
                                     lhsT=onesrow[:, 0:128], rhs=nmrow[:, s5])
                    tmp = tmpp.tile([128, 512], f16, tag="tmp")
                    nc.vector.tensor_tensor(
                        out=tmp, in0=out1[:, s5],
                        in1=bc_ps[:, 0:512], op=OP.mult)
                    nc.vector.tensor_tensor(out=n2c[:, l5], in0=tmp,
                                            in1=bc_ps[:, 512:1024], op=OP.add)
                o_ps = ps_o.tile([128, 1024], f32, tag="ops")
                for j in range(4):
                    h_ps = ps_big.tile([128, 1024], f32, tag="big")
                    w1j = w1_sb[i][:, j * 128:(j + 1) * 128]
                    b1row = b1r_sb[:, (i * 4 + j) * 128:(i * 4 + j + 1) * 128]
                    nc.tensor.matmul(h_ps[:, 0:512], lhsT=w1j,
                                     rhs=n2c[:, 0:512], start=True, stop=False)
                    nc.tensor.matmul(h_ps[:, 0:512], lhsT=b1row,
                                     rhs=onesrow[:, 0:512],
                                     start=False, stop=True)
                    nc.tensor.matmul(h_ps[:, 512:1024], lhsT=w1j,
                                     rhs=n2c[:, 512:1024],
                                     start=True, stop=False)
                    nc.tensor.matmul(h_ps[:, 512:1024], lhsT=b1row,
                                     rhs=onesrow[:, 0:512],
                                     start=False, stop=True)
                    u = tmpp.tile([128, 1024], f16, tag="u")
                    nc.scalar.activation(u, h_ps, AF.Exp)
                    w = tmpp.tile([128, 1024], f32, tag="w")
                    nc.scalar.activation(w, u, AF.Square, bias=1.0)
                    hsb = tmpp.tile([128, 1024], f16, tag="hsb")
                    if MISH_EXACT:
                        den = tmpp.tile([128, 1024], f32, tag="den")
                        nc.vector.tensor_scalar(out=den, in0=w, scalar1=1.0,
                                                scalar2=None, op0=OP.add)
                        r_ = tmpp.tile([128, 1024], f32, tag="r_")
                        nc.vector.reciprocal_approx_fast(r_, den)
                        t_ = tmpp.tile([128, 1024], f16, tag="t_")
                        nc.vector.tensor_scalar(out=t_, in0=r_, scalar1=-2.0,
                                                scalar2=1.0, op0=OP.mult,
                                                op1=OP.add)
                        nc.vector.scalar_tensor_tensor(
                            out=hsb, in0=h_ps, scalar=0.0, in1=t_,
                            op0=OP.add, op1=OP.mult)
                    else:
                        _emit_misht(nc, hsb, h_ps, w)
                    nc.tensor.matmul(o_ps[:, 0:512],
                                     lhsT=w2_sb[i][:, j * 128:(j + 1) * 128],
                                     rhs=hsb[:, 0:512],
                                     start=(j == 0), stop=(j == 3))
                    nc.tensor.matmul(o_ps[:, 512:1024],
                                     lhsT=w2_sb[i][:, j * 128:(j + 1) * 128],
                                     rhs=hsb[:, 512:1024],
                                     start=(j == 0), stop=(j == 3))
                nc.vector.scalar_tensor_tensor(
                    out=out2[:, cs2], in0=o_ps, scalar=b2_sb[:, i:i + 1],
                    in1=out1[:, cs2], op0=OP.add, op1=OP.add)
            return out2

        # ---- main item loop ------------------------------------------
        with tc.For_i(0, n_items, 1, staggered_reset=True) as it:
            x1 = bigp.tile([128, S], f16, tag="x1")
            x2 = bigp.tile([128, S], f16, tag="x2")
            xi = xin[bass.ds(it, 1)].squeeze(0)
            nc.sync.dma_start(out=x1, in_=xi[0:128])
            nc.sync.dma_start(out=x2, in_=xi[128:256])
            if use_mask:
                mb_sb = tinyp.tile([DM, HE], f32, tag="mb")
                nc.sync.dma_start(out=mb_sb,
                                  in_=mb_d[bass.ds(it, 1)].squeeze(0))
            else:
                mb_sb = mb0_sb

            sA, sB = x1, x2
            for i in range(L):
                sA = emit_layer(i, sA, sA, mb_sb)
                sB = emit_layer(i, sB, sB, mb_sb)
            nc.sync.dma_start(out=m1_o[bass.ds(it, 1)].squeeze(0), in_=sA)
            nc.sync.dma_start(out=m2_o[bass.ds(it, 1)].squeeze(0), in_=sB)

            eA, eB = x2, x1
            for i in range(L):
                kvA = x1 if i == 0 else eA
                kvB = x2 if i == 0 else eB
                eA = emit_layer(i, eA, kvA, mb_sb)
                eB = emit_layer(i, eB, kvB, mb_sb)

            enc = statep.tile([128, S], f16, tag="state")
            for c in range(4):
                cs = slice(c * 512, (c + 1) * 512)
                en_ps = ps_big.tile([128, 512], f32, tag="big")
                nc.tensor.matmul(en_ps, lhsT=wl2a, rhs=eA[:, cs],
                                 start=True, stop=False)
                nc.tensor.matmul(en_ps, lhsT=wl2b, rhs=eB[:, cs],
                                 start=False, stop=True)
                nc.vector.tensor_scalar(out=enc[:, cs], in0=en_ps,
                                        scalar1=bl2_sb, scalar2=None,
                                        op0=OP.add)
            for i in range(L):
                enc = emit_layer(i, enc, enc, mb_sb)
            nc.sync.dma_start(out=e_o[bass.ds(it, 1)].squeeze(0), in_=enc)

    nc.compile()
    return nc


_CACHE = {}


def _get_built(n_items, use_bq, use_mask):
    key = (n_items, use_bq, use_mask)
    if key not in _CACHE:
        _CACHE[key] = build(n_items, use_bq, use_mask)
    return _CACHE[key]


def _in_maps(f, src16, n_items, n_cores, use_bq, use_mask):
    base = {
        'wq': f['wq'], 'ncq': f['ncq'].reshape(1, L * DM), 'wk': f['wk'],
        'wv': f['wv'], 'wfc': f['wfc'], 'w1': f['w1'], 'b1r': f['b1r'],
        'w2r': f['w2r'], 'b2s': f['b2s'], 'wl2': f['wl2'], 'bl2': f['bl2'],
        'crow': np.ones((1, DI), np.float16),
    }
    if use_bq:
        base['bqr'] = f['bqr'].reshape(1, L * DM)
    if not use_mask:
        base['mb0'] = f['mb0']
    maps = []
    for c in range(n_cores):
        m = dict(base)
        m['xin'] = src16[c * n_items:(c + 1) * n_items]
        if use_mask:
            m['mb'] = np.ascontiguousarray(
                f['mb'][c * n_items:(c + 1) * n_items])
        maps.append(m)
    return maps


def run(inputs, trace=False):
    from concourse import bass_utils
    from concourse.bass_utils import run_bass_kernel_spmd
    if trace:
        import ntff_shim
        ntff_shim.install()
        bass_utils.upload_artifacts = lambda tmpdir: tmpdir
    f = fold_weights(inputs)
    use_bq = not f['bq_trivial']
    use_mask = not f['mask_trivial']
    src = np.asarray(inputs['src_seq'], np.float32)
    nb = src.shape[0]
    n_cores = N_CORES if nb % N_CORES == 0 else 1
    n_items = nb // n_cores
    # host: feature-major fp16
    src16 = np.ascontiguousarray(src.transpose(0, 2, 1).astype(np.float16))
    nc = _get_built(n_items, use_bq, use_mask)
    maps = _in_maps(f, src16, n_items, n_cores, use_bq, use_mask)
    res = run_bass_kernel_spmd(nc, maps, core_ids=list(range(n_cores)),
                               trace=trace, trace_cores=[0] if trace else None)

    def gather(name):
        parts = [res.results[c][name] for c in range(n_cores)]
        full = np.concatenate(parts, 0)          # [B, 128, S] f16
        return np.ascontiguousarray(
            full.transpose(0, 2, 1).astype(np.float32))
    return (gather('eo'), gather('m1o'), gather('m2o')), res


def kernel(**inputs):
    (enc, m1, m2), _ = run(inputs, trace=False)
    return (enc, m1, m2)


# revision 10
# speedup vs baseline: 1.1639x; 1.1639x over previous
"""Trainium2 Bass kernel for nn_Encoder_45475113730366 (v2).

Data-parallel over batch (64 -> 8 cores x 8 items). Per item the 4-layer
encoder stack runs on 5 streams (m1, m2, e1, e2, enc).

v2 design (vs v1 baseline):
  - fp16 everywhere on device (weights, activations, states, DRAM IO);
    PSUM accumulation stays fp32.  Host pre-transposes the input to
    feature-major [item, 256, 2048] fp16 and post-transposes the outputs,
    so there are no on-device layout transposes for IO.
  - q-projection eliminated: scores S = wq^T P - cq (x) s with
    P = x^T (rstd*k), s = colmean(P).  LN1 mean/rstd fold algebraically.
  - softmax fused to 7 ops via 3D-AP reduces + stride-0 broadcast TT.
  - attention applied as one [64->128] projection C = bda^T wfcP; the
    block-diagonal attn weights come from masking scores with -1e9.
  - FFN bias b1 folded into h_ps via rank-1 matmuls (b1_j (x) ones).
  - Mish tail = ONE custom DVE op (seed+Chebyshev-NR reciprocal fused
    with the final multiply), after ACT Exp + ACT Square.
"""
from contextlib import ExitStack

import numpy as np

import concourse.bacc as bacc
import concourse.bass as bass
import concourse.tile as tile
from concourse import mybir
from concourse.masks import make_identity

N_CORES = 8
B, S, DM, H, DK, DI, L = 64, 2048, 128, 8, 16, 512, 4
DKP = DK // 2
HE = H * DKP          # 64 pooled kv features
NT = S // 128         # 16 token tiles
EPS = 1e-6
TEMP = 0.5 * float(np.sqrt(DK))
QK = 0x5f3759df       # quake rsqrt seed constant

f32 = mybir.dt.float32
f16 = mybir.dt.float16
i32 = mybir.dt.int32
AX = mybir.AxisListType.X
OP = mybir.AluOpType
AF = mybir.ActivationFunctionType

# ---------------------------------------------------------------------------
# custom fused DVE op: mish tail
#   out = Src1 * (1 - y1), y1 ~= 2/(Src0+1) via ~bits seed + Chebyshev-NR.
#   Src0 = w = (1+e^hb)^2 (fp32), Src1 = hb (fp32 PSUM, b1 included).
MC1 = -0.33699572
MC2 = 2.83013085


def _ref_misht(in0, in1, s0, s1, imm2):
    den = (in0.astype(np.float32) + 1.0).astype(np.float32)
    nx = (~den.view(np.uint32)).view(np.float32)
    y0 = nx * np.float32(s1)
    y1 = y0 * (np.float32(imm2) - den * y0)
    return (in1.astype(np.float32) * (1.0 - y1)).astype(np.float32)


def _register_misht():
    import concourse.dve_ops as dv
    from concourse.dve_spec import Spec, Src0, Src1, C1, C2, One, lower
    from concourse.dve_ops import DveOp, Bin
    from concourse.dve_uop import AluOp, DveOpSpec

    if "MISH_T_ANT" in dv._SUB_OPCODE_FOR_NAME:
        return next(o for o in dv.OPS if o.name == "MISH_T_ANT")
    den = Src0 + One
    nx = Bin(AluOp.BITWISE_NOT, den, den)
    y0 = nx * C1
    y1 = y0 * (C2 - den * y0)
    op = DveOp("MISH_T_ANT", Spec(body=Src1 * (One - y1), reference=_ref_misht),
               subdim=False, uops_sha={})
    opc = max(dv._SUB_OPCODE_FOR_NAME.values()) + 1
    assert opc < 0x20
    for ver in ("v3", "v4"):
        try:
            uops = lower(op.spec, ver=ver)
        except Exception:
            continue
        op.uops_sha[ver] = DveOpSpec(name=op.name, opcode=opc, uops=uops).sha(ver)
    dv.OPS.append(op)
    dv.CUSTOM_DVE_SPECS[op.name] = op.spec
    dv._SUB_OPCODE_FOR_NAME[op.name] = opc
    return op


def _emit_misht(nc, out, h_ps, w_sb):
    op = _register_misht()
    return nc.vector._custom_dve(op, out=out, in0=w_sb, in1=h_ps,
                                 s0=0.0, s1=MC1, imm2=MC2)


# ---------------------------------------------------------------------------
def fold_weights(inp):
    f = {}
    Wq = np.asarray(inp['Wq'], np.float32)
    Wk = np.asarray(inp['Wk'], np.float32)
    Wv = np.asarray(inp['Wv'], np.float32)
    Wfc = np.asarray(inp['Wfc'], np.float32)
    W1 = np.asarray(inp['W1'], np.float32)
    W2 = np.asarray(inp['W2'], np.float32)
    g1 = np.asarray(inp['ln1_g'], np.float32)
    b1n = np.asarray(inp['ln1_b'], np.float32)
    g2 = np.asarray(inp['ln2_g'], np.float32)
    b2n = np.asarray(inp['ln2_b'], np.float32)

    wq = ((g1[:, :, None] * Wq) / TEMP).astype(np.float16)       # [L,128,128]
    f['wq'] = wq
    # rank-1 mean-correction row: ncq = -colsum(wq) (fp16-consistent)
    f['ncq'] = (-wq.astype(np.float32).sum(axis=1)).astype(np.float16) \
        .reshape(L, DM)                                          # [L,128]
    bq = np.einsum('ld,ldf->lf', b1n, Wq) / TEMP
    f['bq_trivial'] = bool(np.abs(bq).max() == 0.0)
    f['bqr'] = bq.astype(np.float16)                             # [L,128]
    f['wk'] = Wk.reshape(L, DM, H, DKP, 2).mean(-1).reshape(L, DM, HE) \
        .astype(np.float16)
    f['wv'] = Wv.reshape(L, DM, H, DKP, 2).mean(-1).reshape(L, DM, HE) \
        .astype(np.float16)
    perm = np.array([d * H + h for h in range(H) for d in range(DK)])
    f['wfc'] = Wfc[:, perm, :].astype(np.float16)                # [L,128,128]
    f['w1'] = (g2[:, :, None] * W1).astype(np.float16)           # [L,128,512]
    b1f = np.einsum('ld,ldf->lf', b2n, W1) + np.asarray(inp['b1'], np.float32)
    # rank-1 bias rows, flattened [1, L*4*128]
    f['b1r'] = b1f.reshape(1, L * DI).astype(np.float16)
    # W2 rearranged: chunk j is a [128,128] lhsT at cols j*128:(j+1)*128
    f['w2r'] = W2.reshape(L, 4, 128, DM).transpose(0, 2, 1, 3) \
        .reshape(L, 128, 4 * DM).astype(np.float16)
    f['b2s'] = np.ascontiguousarray(
        np.asarray(inp['b2'], np.float32).T)                     # [128,L]
    f['wl2'] = np.asarray(inp['WL2'], np.float32).astype(np.float16)
    f['bl2'] = np.asarray(inp['bL2'], np.float32).reshape(DM, 1)  # [128,1]

    mask = np.asarray(inp['src_mask'])                           # [B,16,8]
    f['mask_trivial'] = bool(mask.all())
    # score-space mask bias [*, 128, 64]: row p=(h,d), col c=(h',e):
    # 0 where h'==h and mask[d,e], else -1e9
    blk = np.full((DM, HE), -1e9, np.float32)
    mb_all = np.broadcast_to(blk, (mask.shape[0], DM, HE)).copy()
    for h in range(H):
        # on-block region rows h*16:(h+1)*16, cols h*8:(h+1)*8
        sub = mb_all[:, h * DK:(h + 1) * DK, h * DKP:(h + 1) * DKP]
        sub[:] = np.where(mask, 0.0, -1e9)[:, :, :]
    f['mb'] = mb_all                                             # [B,128,64]
    f['mb0'] = mb_all[0]                                         # [128,64]
    return f


# ---------------------------------------------------------------------------
def build(n_items, use_bq, use_mask):
    nc = bacc.Bacc(trn_type="TRN2", target_bir_lowering=False, debug=False)
    _register_misht()

    xin = nc.dram_tensor("xin", [n_items, 2 * DM, S], f16,
                         kind="ExternalInput").ap()
    wq_d = nc.dram_tensor("wq", [L, DM, DM], f16, kind="ExternalInput").ap()
    ncq_d = nc.dram_tensor("ncq", [1, L * DM], f16, kind="ExternalInput").ap()
    wk_d = nc.dram_tensor("wk", [L, DM, HE], f16, kind="ExternalInput").ap()
    wv_d = nc.dram_tensor("wv", [L, DM, HE], f16, kind="ExternalInput").ap()
    wfc_d = nc.dram_tensor("wfc", [L, DM, DM], f16, kind="ExternalInput").ap()
    w1_d = nc.dram_tensor("w1", [L, DM, DI], f16, kind="ExternalInput").ap()
    b1r_d = nc.dram_tensor("b1r", [1, L * DI], f16, kind="ExternalInput").ap()
    w2_d = nc.dram_tensor("w2r", [L, DM, DI], f16, kind="ExternalInput").ap()
    b2_d = nc.dram_tensor("b2s", [DM, L], f32, kind="ExternalInput").ap()
    wl2_d = nc.dram_tensor("wl2", [2 * DM, DM], f16, kind="ExternalInput").ap()
    bl2_d = nc.dram_tensor("bl2", [DM, 1], f32, kind="ExternalInput").ap()
    crow_d = nc.dram_tensor("crow", [1, DI], f16, kind="ExternalInput").ap()
    if use_mask:
        mb_d = nc.dram_tensor("mb", [n_items, DM, HE], f32,
                              kind="ExternalInput").ap()
    else:
        mb_d = nc.dram_tensor("mb0", [DM, HE], f32, kind="ExternalInput").ap()
    if use_bq:
        bqr_d = nc.dram_tensor("bqr", [1, L * DM], f16,
                               kind="ExternalInput").ap()
    m1_o = nc.dram_tensor("m1o", [n_items, DM, S], f16,
                          kind="ExternalOutput").ap()
    m2_o = nc.dram_tensor("m2o", [n_items, DM, S], f16,
                          kind="ExternalOutput").ap()
    e_o = nc.dram_tensor("eo", [n_items, DM, S], f16,
                         kind="ExternalOutput").ap()

    with tile.TileContext(nc) as tc, ExitStack() as ctx:
        consts = ctx.enter_context(tc.tile_pool(name="consts", bufs=1))
        bigp = ctx.enter_context(tc.tile_pool(name="bigp", bufs=1))
        statep = ctx.enter_context(tc.tile_pool(name="statep", bufs=4))
        workp = ctx.enter_context(tc.tile_pool(name="workp", bufs=2))
        tmpp = ctx.enter_context(tc.tile_pool(name="tmpp", bufs=2))
        tinyp = ctx.enter_context(tc.tile_pool(name="tinyp", bufs=3))
        rowp = ctx.enter_context(tc.tile_pool(name="rowp", bufs=2))
        ps_tiny = ctx.enter_context(
            tc.tile_pool(name="ps_tiny", bufs=2, space="PSUM"))
        ps_big = ctx.enter_context(
            tc.tile_pool(name="ps_big", bufs=2, space="PSUM"))
        ps_o = ctx.enter_context(
            tc.tile_pool(name="ps_o", bufs=1, space="PSUM"))

        # ---- constants / weights ------------------------------------
        identf = consts.tile([128, 128], f32, tag="identf")
        make_identity(nc, identf)
        ident16 = consts.tile([128, 128], f16, tag="ident16")
        nc.vector.tensor_copy(ident16, identf)
        ones128 = consts.tile([128, 1], f16, tag="ones128")
        nc.vector.memset(ones128, 1.0 / 128.0)
        ones128f = consts.tile([128, 1], f32, tag="ones128f")
        nc.vector.memset(ones128f, 1.0 / 128.0)
        ln2col = consts.tile([128, 1], f32, tag="ln2col")
        nc.vector.memset(ln2col, 0.6931471805599453)
        onesrow = consts.tile([1, DI], f16, tag="onesrow")
        nc.sync.dma_start(out=onesrow, in_=crow_d)

        def _load(name, dram_ap, shape, dt=f16):
            t = consts.tile(list(shape), dt, tag=name)
            nc.sync.dma_start(out=t, in_=dram_ap)
            return t

        wq_sb = [_load(f"wq{i}", wq_d[i], [128, DM]) for i in range(L)]
        wqf_sb = []
        for i in range(L):
            t = consts.tile([128, DM], f32, tag=f"wqf{i}")
            nc.vector.tensor_copy(t, wq_sb[i])
            wqf_sb.append(t)
        wk_sb = [_load(f"wk{i}", wk_d[i], [128, HE]) for i in range(L)]
        wv_sb = [_load(f"wv{i}", wv_d[i], [128, HE]) for i in range(L)]
        wfc_sb = [_load(f"wfc{i}", wfc_d[i], [128, DM]) for i in range(L)]
        w1_sb = [_load(f"w1{i}", w1_d[i], [128, DI]) for i in range(L)]
        w2_sb = [_load(f"w2{i}", w2_d[i], [128, DI]) for i in range(L)]
        ncq_sb = _load("ncq", ncq_d, [1, L * DM])
        ncqf_sb = consts.tile([1, L * DM], f32, tag="ncqf")
        nc.vector.tensor_copy(ncqf_sb, ncq_sb)
        b1r_sb = _load("b1r", b1r_d, [1, L * DI])
        b2_sb = _load("b2s", b2_d, [DM, L], f32)
        wl2a = _load("wl2a", wl2_d[0:DM], [128, DM])
        wl2b = _load("wl2b", wl2_d[DM:2 * DM], [128, DM])
        bl2_sb = _load("bl2", bl2_d, [128, 1], f32)
        if use_bq:
            bqr_sb = _load("bqr", bqr_d, [1, L * DM])
        if not use_mask:
            mb0_sb = _load("mb0", mb_d, [DM, HE], f32)

        def ap3(t, d1, d2, psrc=None):
            """view SBUF/PSUM 2D tile [128, d1*d2] as [128, d1, d2]"""
            src = psrc if psrc is not None else t
            return bass.AP(tensor=src.tensor, offset=src.offset,
                           ap=[src.ap[0], [d2, d1], [1, d2]])

        def bcast3(t, d1, d2):
            """[128, d1] -> [128, d1, d2] stride-0 broadcast"""
            return bass.AP(tensor=t.tensor, offset=t.offset,
                           ap=[t.ap[0], [1, d1], [0, d2]])

        # ---- per-layer emission --------------------------------------
        def rsqrt_neg(v):
            """[-rstd] via quake seed + 3 Newton iters, [128,16] f32."""
            yi = tinyp.tile([128, 16], i32, tag="yi")
            nc.vector.tensor_scalar(out=yi, in0=v.bitcast(i32), scalar1=1,
                                    scalar2=None, op0=OP.arith_shift_right)
            nc.vector.tensor_scalar(out=yi, in0=yi, scalar1=-1,
                                    scalar2=None, op0=OP.bitwise_xor)
            nc.vector.tensor_scalar(out=yi, in0=yi, scalar1=QK + 1,
                                    scalar2=None, op0=OP.add)
            y = yi.bitcast(f32)
            hv = tinyp.tile([128, 16], f32, tag="hv")
            nc.vector.tensor_scalar(out=hv, in0=v, scalar1=0.5, scalar2=None,
                                    op0=OP.mult)
            tq = tinyp.tile([128, 16], f32, tag="tq")
            for _ in range(3):
                nc.vector.tensor_mul(tq, y, y)
                nc.vector.tensor_mul(tq, tq, hv)
                nc.vector.scalar_tensor_tensor(out=y, in0=tq, scalar=1.5, in1=y,
                                               op0=OP.subtract, op1=OP.mult)
            return y

        def stats(xtok):
            """token-major xtok [128,2048] -> (mu, e2) [128,16] f32 SBUF."""
            mu = tinyp.tile([128, 16], f32, tag="mu")
            nc.vector.tensor_reduce(out=mu, in_=ap3(xtok, NT, 128), axis=AX,
                                    op=OP.add)
            nc.vector.tensor_scalar(out=mu, in0=mu, scalar1=1.0 / 128.0,
                                    scalar2=None, op0=OP.mult)
            sq = tmpp.tile([128, S], f16, tag="sqt")
            for c in range(4):
                nc.gpsimd.tensor_mul(sq[:, c * 512:(c + 1) * 512],
                                     xtok[:, c * 512:(c + 1) * 512],
                                     xtok[:, c * 512:(c + 1) * 512])
            e2 = tinyp.tile([128, 16], f32, tag="e2")
            nc.vector.tensor_reduce(out=e2, in_=ap3(sq, NT, 128), axis=AX,
                                    op=OP.add)
            return mu, e2

        def chain_nrstd(mu, e2):
            """(mu, e2) -> -rstd [128,16] f32 (e2 is a raw sum of squares)."""
            musq = tinyp.tile([128, 16], f32, tag="musq")
            nc.vector.tensor_mul(musq, mu, mu)
            vpe = tinyp.tile([128, 16], f32, tag="vpe")
            nc.vector.scalar_tensor_tensor(out=vpe, in0=e2,
                                           scalar=1.0 / 128.0,
                                           in1=musq, op0=OP.mult,
                                           op1=OP.subtract)
            nc.vector.tensor_scalar(out=vpe, in0=vpe, scalar1=float(EPS),
                                    scalar2=None, op0=OP.add)
            return rsqrt_neg(vpe)

        def tokmajor(x, eng=None):
            """feature-major [128,S] -> per-tile token-major via one DMA.
            out[p, t*128+f] = x[f, t*128+p]."""
            xt = workp.tile([128, NT, 128], f16, tag="xtok")
            (eng or nc.sync).dma_start_transpose(out=xt, in_=x)
            return xt.rearrange("p a b -> p (a b)")

        def emit_layer(i, xq, xkv, mb_sb, deep=False):
            # ---- token-major xq + LN1 stats + k projection ------------
            xqtok = tokmajor(xq)
            mu1, e21 = stats(xqtok)
            k_ps = ps_big.tile([128, 1024], f32, tag="big")
            for t in range(NT):
                nc.tensor.matmul(k_ps[:, t * HE:(t + 1) * HE],
                                 lhsT=xkv[:, t * 128:(t + 1) * 128],
                                 rhs=wk_sb[i])
            ty = ps_tiny.tile([128, 512], f32, tag="ty")
            nrstd = chain_nrstd(mu1, e21)
            rstd = tinyp.tile([128, 16], f32, tag="rstd")
            nc.vector.tensor_scalar(out=rstd, in0=nrstd, scalar1=-1.0,
                                    scalar2=None, op0=OP.mult)

            # ---- k scale (ACT, per token tile) ------------------------
            k_sb = workp.tile([128, NT * HE], f16, tag="ksb")
            for t in range(NT):
                nc.scalar.activation(k_sb[:, t * HE:(t + 1) * HE],
                                     k_ps[:, t * HE:(t + 1) * HE],
                                     AF.Identity, scale=rstd[:, t:t + 1])

            # ---- P accumulation + scores ------------------------------
            p_ps = ty[:, 0:HE]
            for t in range(NT):
                nc.tensor.matmul(p_ps,
                                 lhsT=xqtok[:, t * 128:(t + 1) * 128],
                                 rhs=k_sb[:, t * HE:(t + 1) * HE],
                                 start=(t == 0), stop=(t == NT - 1))
            p_sb = tinyp.tile([128, HE], f16, tag="psb")
            nc.vector.tensor_copy(p_sb, p_ps)
            s_ps = ty[0:1, HE:2 * HE]
            nc.tensor.matmul(s_ps, lhsT=ones128, rhs=p_sb)
            s_sb = tinyp.tile([1, HE], f16, tag="ssb")
            nc.vector.tensor_copy(s_sb, s_ps)
            sc_ps = ty[:, 2 * HE:3 * HE]
            nc.tensor.matmul(sc_ps, lhsT=wq_sb[i], rhs=p_sb,
                             start=True, stop=False)
            ncq_row = ncq_sb[:, i * DM:(i + 1) * DM]
            nc.tensor.matmul(sc_ps, lhsT=ncq_row, rhs=s_sb,
                             start=False, stop=(not use_bq))
            if use_bq:
                xsum = tinyp.tile([128, 1], f32, tag="xsum")
                nc.vector.tensor_reduce(out=xsum, in_=ap3(xkv, 4, 512),
                                        axis=mybir.AxisListType.XY, op=OP.add)
                xsum16 = tinyp.tile([128, 1], f16, tag="xsum16")
                nc.vector.tensor_copy(xsum16, xsum)
                ks_ps = ty[0:1, 3 * HE:4 * HE]
                nc.tensor.matmul(ks_ps, lhsT=xsum16, rhs=wk_sb[i])
                ks_sb = tinyp.tile([1, HE], f16, tag="kss")
                nc.vector.tensor_copy(ks_sb, ks_ps)
                nc.tensor.matmul(sc_ps, lhsT=bqr_sb[:, i * DM:(i + 1) * DM],
                                 rhs=ks_sb, start=False, stop=True)

            # ---- softmax -> block-diagonal attn (fp16) ----------------
            sm = tinyp.tile([128, HE], f32, tag="sm")
            nc.vector.tensor_tensor(out=sm, in0=sc_ps, in1=mb_sb, op=OP.add)
            negmx = tinyp.tile([128, 1], f32, tag="negmx")
            nc.vector.tensor_reduce(out=negmx, in_=sm, axis=AX,
                                    op=OP.max, negate=True)
            sm2 = tinyp.tile([128, HE], f32, tag="sm2")
            nc.vector.tensor_scalar(out=sm2, in0=sm, scalar1=negmx,
                                    scalar2=None, op0=OP.add)
            es = tinyp.tile([128, HE], f32, tag="es")
            nc.scalar.activation(es, sm2, AF.Exp)
            ssum = tinyp.tile([128, H], f32, tag="ssum")
            nc.vector.tensor_reduce(out=ssum, in_=ap3(es, H, DKP), axis=AX,
                                    op=OP.add)
            nc.vector.tensor_scalar(out=ssum, in0=ssum, scalar1=1e-30,
                                    scalar2=None, op0=OP.add)
            rs = tinyp.tile([128, H], f32, tag="rs")
            nc.vector.reciprocal(rs, ssum)
            bda = tinyp.tile([128, HE], f16, tag="bda")
            nc.vector.tensor_tensor(out=ap3(bda, H, DKP), in0=ap3(es, H, DKP),
                                    in1=bcast3(rs, H, DKP), op=OP.mult)

            # ---- C = bda^T wfcP ---------------------------------------
            c_ps = ty[0:HE, 4 * HE:4 * HE + 128]
            nc.tensor.matmul(c_ps, lhsT=bda, rhs=wfc_sb[i])
            c_sb = tinyp.tile([HE, 128], f16, tag="csb")
            nc.vector.tensor_copy(c_sb, c_ps)

            # ---- v projection + attn out + residual -------------------
            vT = workp.tile([HE, S], f16, tag="vt")
            for c in range(4):
                v_ps = ps_big.tile([HE, 512], f32, tag="big")
                nc.tensor.matmul(v_ps, lhsT=wv_sb[i],
                                 rhs=xkv[:, c * 512:(c + 1) * 512])
                nc.scalar.copy(out=vT[:, c * 512:(c + 1) * 512], in_=v_ps)
            out1 = workp.tile([128, S], f16, tag="o1")
            for c in range(4):
                cs = slice(c * 512, (c + 1) * 512)
                ao_ps = ps_big.tile([128, 512], f32, tag="big")
                nc.tensor.matmul(ao_ps, lhsT=c_sb, rhs=vT[:, cs])
                nc.vector.tensor_tensor(out=out1[:, cs], in0=ao_ps,
                                        in1=xq[:, cs], op=OP.add)

            # ---- LN2 stats + rows -------------------------------------
            o1tok = tokmajor(out1)
            mu2, e22 = stats(o1tok)
            nrstd2 = chain_nrstd(mu2, e22)
            r2h = tinyp.tile([128, 16], f16, tag="r2h")
            nc.vector.tensor_scalar(out=r2h, in0=nrstd2, scalar1=-1.0,
                                    scalar2=None, op0=OP.mult)
            nm2h = tinyp.tile([128, 16], f16, tag="nm2h")
            nc.vector.tensor_tensor(out=nm2h, in0=mu2, in1=nrstd2, op=OP.mult)
            tr_ps = ty[0:16, 384:512].bitcast(f16)
            nc.tensor.transpose(tr_ps[:, 0:128], r2h, ident16)
            nc.tensor.transpose(tr_ps[:, 128:256], nm2h, ident16)
            rows = rowp.tile([16, 256], f16, tag="rows")
            nc.vector.tensor_copy(rows, tr_ps)
            rowrow = rowp.tile([1, 2 * S], f16, tag="rr")
            r2row = rowrow[:, 0:S]
            nmrow = rowrow[:, S:2 * S]
            nc.sync.dma_start(out=r2row, in_=rows[:, 0:128])
            nc.sync.dma_start(out=nmrow, in_=rows[:, 128:256])

            # ---- FFN per 1024-token chunk -----------------------------
            out2 = statep.tile([128, S], f16, tag="state")
            for c2 in range(2):
                cs2 = slice(c2 * 1024, (c2 + 1) * 1024)
                n2c = tmpp.tile([128, 1024], f16, tag="n2c")
                for cc in range(2):
                    c = 2 * c2 + cc
                    s5 = slice(c * 512, (c + 1) * 512)
                    l5 = slice(cc * 512, (cc + 1) * 512)
                    bc_ps = ps_big.tile([128, 1024], f32, tag="big")
                    nc.tensor.matmul(bc_ps[:, 0:512],
                                     lhsT=onesrow[:, 0:128], rhs=r2row[:, s5])
                    nc.tensor.matmul(bc_ps[:, 512:1024],
                                     lhsT=onesrow[:, 0:128], rhs=nmrow[:, s5])
                    tmp = tmpp.tile([128, 512], f16, tag="tmp")
                    nc.vector.tensor_tensor(
                        out=tmp, in0=out1[:, s5],
                        in1=bc_ps[:, 0:512], op=OP.mult)
                    nc.vector.tensor_tensor(out=n2c[:, l5], in0=tmp,
                                            in1=bc_ps[:, 512:1024], op=OP.add)
                o_ps = ps_o.tile([128, 1024], f32, tag="ops")
                for j in range(4):
                    h_ps = ps_big.tile([128, 1024], f32, tag="big")
                    w1j = w1_sb[i][:, j * 128:(j + 1) * 128]
                    b1row = b1r_sb[:, (i * 4 + j) * 128:(i * 4 + j + 1) * 128]
                    nc.tensor.matmul(h_ps[:, 0:512], lhsT=w1j,
                                     rhs=n2c[:, 0:512], start=True, stop=False)
                    nc.tensor.matmul(h_ps[:, 0:512], lhsT=b1row,
                                     rhs=onesrow[:, 0:512],
                                     start=False, stop=True)
                    nc.tensor.matmul(h_ps[:, 512:1024], lhsT=w1j,
                                     rhs=n2c[:, 512:1024],
                                     start=True, stop=False)
                    nc.tensor.matmul(h_ps[:, 512:1024], lhsT=b1row,
                                     rhs=onesrow[:, 0:512],
                                     start=False, stop=True)
                    hsb = tmpp.tile([128, 1024], f16, tag="hsb")
                    if deep:
                        u = tmpp.tile([128, 1024], f32, tag="u32")
                        nc.scalar.activation(u, h_ps, AF.Exp)
                        w = tmpp.tile([128, 1024], f32, tag="w")
                        nc.scalar.activation(w, u, AF.Square, bias=1.0)
                        q_ = tmpp.tile([128, 1024], f32, tag="q_")
                        nc.scalar.activation(q_, w, AF.Ln, bias=1.0)
                        r_ = tmpp.tile([128, 1024], f32, tag="r_")
                        nc.scalar.activation(r_, q_, AF.Exp, scale=-1.0,
                                             bias=ln2col)
                        t_ = tmpp.tile([128, 1024], f16, tag="t_")
                        nc.vector.tensor_scalar(out=t_, in0=r_, scalar1=-1.0,
                                                scalar2=1.0, op0=OP.mult,
                                                op1=OP.add)
                        nc.vector.scalar_tensor_tensor(
                            out=hsb, in0=h_ps, scalar=0.0, in1=t_,
                            op0=OP.add, op1=OP.mult)
                    else:
                        u = tmpp.tile([128, 1024], f16, tag="u")
                        nc.scalar.activation(u, h_ps, AF.Exp)
                        w = tmpp.tile([128, 1024], f32, tag="w")
                        nc.scalar.activation(w, u, AF.Square, bias=1.0)
                        _emit_misht(nc, hsb, h_ps, w)
                    nc.tensor.matmul(o_ps[:, 0:512],
                                     lhsT=w2_sb[i][:, j * 128:(j + 1) * 128],
                                     rhs=hsb[:, 0:512],
                                     start=(j == 0), stop=(j == 3))
                    nc.tensor.matmul(o_ps[:, 512:1024],
                                     lhsT=w2_sb[i][:, j * 128:(j + 1) * 128],
                                     rhs=hsb[:, 512:1024],
                                     start=(j == 0), stop=(j == 3))
                nc.vector.scalar_tensor_tensor(
                    out=out2[:, cs2], in0=o_ps, scalar=b2_sb[:, i:i + 1],
                    in1=out1[:, cs2], op0=OP.add, op1=OP.add)
            return out2

        # ---- main item loop ------------------------------------------
        with tc.For_i(0, n_items, 1, staggered_reset=True) as it:
            x1 = bigp.tile([128, S], f16, tag="x1")
            x2 = bigp.tile([128, S], f16, tag="x2")
            xi = xin[bass.ds(it, 1)].squeeze(0)
            nc.sync.dma_start(out=x1, in_=xi[0:128])
            nc.sync.dma_start(out=x2, in_=xi[128:256])
            if use_mask:
                mb_sb = tinyp.tile([DM, HE], f32, tag="mb")
                nc.sync.dma_start(out=mb_sb,
                                  in_=mb_d[bass.ds(it, 1)].squeeze(0))
            else:
                mb_sb = mb0_sb

            sA, sB = x1, x2
            for i in range(L):
                sA = emit_layer(i, sA, sA, mb_sb)
                sB = emit_layer(i, sB, sB, mb_sb)
            nc.sync.dma_start(out=m1_o[bass.ds(it, 1)].squeeze(0), in_=sA)
            nc.sync.dma_start(out=m2_o[bass.ds(it, 1)].squeeze(0), in_=sB)

            eA, eB = x2, x1
            for i in range(L):
                kvA = x1 if i == 0 else eA
                kvB = x2 if i == 0 else eB
                eA = emit_layer(i, eA, kvA, mb_sb, deep=True)
                eB = emit_layer(i, eB, kvB, mb_sb, deep=True)

            enc = statep.tile([128, S], f16, tag="state")
            for c in range(4):
                cs = slice(c * 512, (c + 1) * 512)
                en_ps = ps_big.tile([128, 512], f32, tag="big")
                nc.tensor.matmul(en_ps, lhsT=wl2a, rhs=eA[:, cs],
                                 start=True, stop=False)
                nc.tensor.matmul(en_ps, lhsT=wl2b, rhs=eB[:, cs],
                                 start=False, stop=True)
                nc.vector.tensor_scalar(out=enc[:, cs], in0=en_ps,
                                        scalar1=bl2_sb, scalar2=None,
                                        op0=OP.add)
            for i in range(L):
                enc = emit_layer(i, enc, enc, mb_sb, deep=True)
            nc.sync.dma_start(out=e_o[bass.ds(it, 1)].squeeze(0), in_=enc)

    nc.compile()
    return nc


_CACHE = {}


def _get_built(n_items, use_bq, use_mask):
    key = (n_items, use_bq, use_mask)
    if key not in _CACHE:
        _CACHE[key] = build(n_items, use_bq, use_mask)
    return _CACHE[key]


def _in_maps(f, src16, n_items, n_cores, use_bq, use_mask):
    base = {
        'wq': f['wq'], 'ncq': f['ncq'].reshape(1, L * DM), 'wk': f['wk'],
        'wv': f['wv'], 'wfc': f['wfc'], 'w1': f['w1'], 'b1r': f['b1r'],
        'w2r': f['w2r'], 'b2s': f['b2s'], 'wl2': f['wl2'], 'bl2': f['bl2'],
        'crow': np.ones((1, DI), np.float16),
    }
    if use_bq:
        base['bqr'] = f['bqr'].reshape(1, L * DM)
    if not use_mask:
        base['mb0'] = f['mb0']
    maps = []
    for c in range(n_cores):
        m = dict(base)
        m['xin'] = src16[c * n_items:(c + 1) * n_items]
        if use_mask:
            m['mb'] = np.ascontiguousarray(
                f['mb'][c * n_items:(c + 1) * n_items])
        maps.append(m)
    return maps


def run(inputs, trace=False):
    from concourse import bass_utils
    from concourse.bass_utils import run_bass_kernel_spmd
    if trace:
        import ntff_shim
        ntff_shim.install()
        bass_utils.upload_artifacts = lambda tmpdir: tmpdir
    f = fold_weights(inputs)
    use_bq = not f['bq_trivial']
    use_mask = not f['mask_trivial']
    src = np.asarray(inputs['src_seq'], np.float32)
    nb = src.shape[0]
    n_cores = N_CORES if nb % N_CORES == 0 else 1
    n_items = nb // n_cores
    # host: feature-major fp16
    src16 = np.ascontiguousarray(src.transpose(0, 2, 1).astype(np.float16))
    nc = _get_built(n_items, use_bq, use_mask)
    maps = _in_maps(f, src16, n_items, n_cores, use_bq, use_mask)
    res = run_bass_kernel_spmd(nc, maps, core_ids=list(range(n_cores)),
                               trace=trace, trace_cores=[0] if trace else None)

    def gather(name):
        parts = [res.results[c][name] for c in range(n_cores)]
        full = np.concatenate(parts, 0)          # [B, 128, S] f16
        return np.ascontiguousarray(
            full.transpose(0, 2, 1).astype(np.float32))
    return (gather('eo'), gather('m1o'), gather('m2o')), res


def kernel(**inputs):
    (enc, m1, m2), _ = run(inputs, trace=False)
    return (enc, m1, m2)


# revision 12
# speedup vs baseline: 1.1661x; 1.0019x over previous
"""Trainium2 Bass kernel for nn_Encoder_45475113730366 (v2).

Data-parallel over batch (64 -> 8 cores x 8 items). Per item the 4-layer
encoder stack runs on 5 streams (m1, m2, e1, e2, enc).

v2 design (vs v1 baseline):
  - fp16 everywhere on device (weights, activations, states, DRAM IO);
    PSUM accumulation stays fp32.  Host pre-transposes the input to
    feature-major [item, 256, 2048] fp16 and post-transposes the outputs,
    so there are no on-device layout transposes for IO.
  - q-projection eliminated: scores S = wq^T P - cq (x) s with
    P = x^T (rstd*k), s = colmean(P).  LN1 mean/rstd fold algebraically.
  - softmax fused to 7 ops via 3D-AP reduces + stride-0 broadcast TT.
  - attention applied as one [64->128] projection C = bda^T wfcP; the
    block-diagonal attn weights come from masking scores with -1e9.
  - FFN bias b1 folded into h_ps via rank-1 matmuls (b1_j (x) ones).
  - Mish tail = ONE custom DVE op (seed+Chebyshev-NR reciprocal fused
    with the final multiply), after ACT Exp + ACT Square.
"""
from contextlib import ExitStack

import numpy as np

import concourse.bacc as bacc
import concourse.bass as bass
import concourse.tile as tile
from concourse import mybir
from concourse.masks import make_identity

N_CORES = 8
B, S, DM, H, DK, DI, L = 64, 2048, 128, 8, 16, 512, 4
DKP = DK // 2
HE = H * DKP          # 64 pooled kv features
NT = S // 128         # 16 token tiles
EPS = 1e-6
TEMP = 0.5 * float(np.sqrt(DK))
QK = 0x5f3759df       # quake rsqrt seed constant

f32 = mybir.dt.float32
f16 = mybir.dt.float16
i32 = mybir.dt.int32
AX = mybir.AxisListType.X
OP = mybir.AluOpType
AF = mybir.ActivationFunctionType

# ---------------------------------------------------------------------------
# custom fused DVE op: mish tail
#   out = Src1 * (1 - y1), y1 ~= 2/(Src0+1) via ~bits seed + Chebyshev-NR.
#   Src0 = w = (1+e^hb)^2 (fp32), Src1 = hb (fp32 PSUM, b1 included).
MC1 = -0.33699572
MC2 = 2.83013085


def _ref_misht(in0, in1, s0, s1, imm2):
    den = (in0.astype(np.float32) + 1.0).astype(np.float32)
    nx = (~den.view(np.uint32)).view(np.float32)
    y0 = nx * np.float32(s1)
    y1 = y0 * (np.float32(imm2) - den * y0)
    return (in1.astype(np.float32) * (1.0 - y1)).astype(np.float32)


def _register_misht():
    import concourse.dve_ops as dv
    from concourse.dve_spec import Spec, Src0, Src1, C1, C2, One, lower
    from concourse.dve_ops import DveOp, Bin
    from concourse.dve_uop import AluOp, DveOpSpec

    if "MISH_T_ANT" in dv._SUB_OPCODE_FOR_NAME:
        return next(o for o in dv.OPS if o.name == "MISH_T_ANT")
    den = Src0 + One
    nx = Bin(AluOp.BITWISE_NOT, den, den)
    y0 = nx * C1
    y1 = y0 * (C2 - den * y0)
    op = DveOp("MISH_T_ANT", Spec(body=Src1 * (One - y1), reference=_ref_misht),
               subdim=False, uops_sha={})
    opc = max(dv._SUB_OPCODE_FOR_NAME.values()) + 1
    assert opc < 0x20
    for ver in ("v3", "v4"):
        try:
            uops = lower(op.spec, ver=ver)
        except Exception:
            continue
        op.uops_sha[ver] = DveOpSpec(name=op.name, opcode=opc, uops=uops).sha(ver)
    dv.OPS.append(op)
    dv.CUSTOM_DVE_SPECS[op.name] = op.spec
    dv._SUB_OPCODE_FOR_NAME[op.name] = opc
    return op


def _emit_misht(nc, out, h_ps, w_sb):
    op = _register_misht()
    return nc.vector._custom_dve(op, out=out, in0=w_sb, in1=h_ps,
                                 s0=0.0, s1=MC1, imm2=MC2)


# ---------------------------------------------------------------------------
def fold_weights(inp):
    f = {}
    Wq = np.asarray(inp['Wq'], np.float32)
    Wk = np.asarray(inp['Wk'], np.float32)
    Wv = np.asarray(inp['Wv'], np.float32)
    Wfc = np.asarray(inp['Wfc'], np.float32)
    W1 = np.asarray(inp['W1'], np.float32)
    W2 = np.asarray(inp['W2'], np.float32)
    g1 = np.asarray(inp['ln1_g'], np.float32)
    b1n = np.asarray(inp['ln1_b'], np.float32)
    g2 = np.asarray(inp['ln2_g'], np.float32)
    b2n = np.asarray(inp['ln2_b'], np.float32)

    wq = ((g1[:, :, None] * Wq) / TEMP).astype(np.float16)       # [L,128,128]
    f['wq'] = wq
    # rank-1 mean-correction row: ncq = -colsum(wq) (fp16-consistent)
    f['ncq'] = (-wq.astype(np.float32).sum(axis=1)).astype(np.float16) \
        .reshape(L, DM)                                          # [L,128]
    bq = np.einsum('ld,ldf->lf', b1n, Wq) / TEMP
    f['bq_trivial'] = bool(np.abs(bq).max() == 0.0)
    f['bqr'] = bq.astype(np.float16)                             # [L,128]
    f['wk'] = Wk.reshape(L, DM, H, DKP, 2).mean(-1).reshape(L, DM, HE) \
        .astype(np.float16)
    f['wv'] = Wv.reshape(L, DM, H, DKP, 2).mean(-1).reshape(L, DM, HE) \
        .astype(np.float16)
    perm = np.array([d * H + h for h in range(H) for d in range(DK)])
    f['wfc'] = Wfc[:, perm, :].astype(np.float16)                # [L,128,128]
    f['w1'] = (g2[:, :, None] * W1).astype(np.float16)           # [L,128,512]
    b1f = np.einsum('ld,ldf->lf', b2n, W1) + np.asarray(inp['b1'], np.float32)
    # rank-1 bias rows, flattened [1, L*4*128]
    f['b1r'] = b1f.reshape(1, L * DI).astype(np.float16)
    # W2 rearranged: chunk j is a [128,128] lhsT at cols j*128:(j+1)*128
    f['w2r'] = W2.reshape(L, 4, 128, DM).transpose(0, 2, 1, 3) \
        .reshape(L, 128, 4 * DM).astype(np.float16)
    f['b2s'] = np.ascontiguousarray(
        np.asarray(inp['b2'], np.float32).T)                     # [128,L]
    f['wl2'] = np.asarray(inp['WL2'], np.float32).astype(np.float16)
    f['bl2'] = np.asarray(inp['bL2'], np.float32).reshape(DM, 1)  # [128,1]

    mask = np.asarray(inp['src_mask'])                           # [B,16,8]
    f['mask_trivial'] = bool(mask.all())
    # score-space mask bias [*, 128, 64]: row p=(h,d), col c=(h',e):
    # 0 where h'==h and mask[d,e], else -1e9
    blk = np.full((DM, HE), -1e9, np.float32)
    mb_all = np.broadcast_to(blk, (mask.shape[0], DM, HE)).copy()
    for h in range(H):
        # on-block region rows h*16:(h+1)*16, cols h*8:(h+1)*8
        sub = mb_all[:, h * DK:(h + 1) * DK, h * DKP:(h + 1) * DKP]
        sub[:] = np.where(mask, 0.0, -1e9)[:, :, :]
    f['mb'] = mb_all                                             # [B,128,64]
    f['mb0'] = mb_all[0]                                         # [128,64]
    return f


# ---------------------------------------------------------------------------
def build(n_items, use_bq, use_mask):
    nc = bacc.Bacc(trn_type="TRN2", target_bir_lowering=False, debug=False)
    _register_misht()

    xin = nc.dram_tensor("xin", [n_items, 2 * DM, S], f16,
                         kind="ExternalInput").ap()
    wq_d = nc.dram_tensor("wq", [L, DM, DM], f16, kind="ExternalInput").ap()
    ncq_d = nc.dram_tensor("ncq", [1, L * DM], f16, kind="ExternalInput").ap()
    wk_d = nc.dram_tensor("wk", [L, DM, HE], f16, kind="ExternalInput").ap()
    wv_d = nc.dram_tensor("wv", [L, DM, HE], f16, kind="ExternalInput").ap()
    wfc_d = nc.dram_tensor("wfc", [L, DM, DM], f16, kind="ExternalInput").ap()
    w1_d = nc.dram_tensor("w1", [L, DM, DI], f16, kind="ExternalInput").ap()
    b1r_d = nc.dram_tensor("b1r", [1, L * DI], f16, kind="ExternalInput").ap()
    w2_d = nc.dram_tensor("w2r", [L, DM, DI], f16, kind="ExternalInput").ap()
    b2_d = nc.dram_tensor("b2s", [DM, L], f32, kind="ExternalInput").ap()
    wl2_d = nc.dram_tensor("wl2", [2 * DM, DM], f16, kind="ExternalInput").ap()
    bl2_d = nc.dram_tensor("bl2", [DM, 1], f32, kind="ExternalInput").ap()
    crow_d = nc.dram_tensor("crow", [1, DI], f16, kind="ExternalInput").ap()
    if use_mask:
        mb_d = nc.dram_tensor("mb", [n_items, DM, HE], f32,
                              kind="ExternalInput").ap()
    else:
        mb_d = nc.dram_tensor("mb0", [DM, HE], f32, kind="ExternalInput").ap()
    if use_bq:
        bqr_d = nc.dram_tensor("bqr", [1, L * DM], f16,
                               kind="ExternalInput").ap()
    m1_o = nc.dram_tensor("m1o", [n_items, DM, S], f16,
                          kind="ExternalOutput").ap()
    m2_o = nc.dram_tensor("m2o", [n_items, DM, S], f16,
                          kind="ExternalOutput").ap()
    e_o = nc.dram_tensor("eo", [n_items, DM, S], f16,
                         kind="ExternalOutput").ap()

    with tile.TileContext(nc) as tc, ExitStack() as ctx:
        consts = ctx.enter_context(tc.tile_pool(name="consts", bufs=1))
        bigp = ctx.enter_context(tc.tile_pool(name="bigp", bufs=1))
        statep = ctx.enter_context(tc.tile_pool(name="statep", bufs=5))
        workp = ctx.enter_context(tc.tile_pool(name="workp", bufs=3))
        tmpp = ctx.enter_context(tc.tile_pool(name="tmpp", bufs=2))
        tinyp = ctx.enter_context(tc.tile_pool(name="tinyp", bufs=3))
        rowp = ctx.enter_context(tc.tile_pool(name="rowp", bufs=2))
        ps_tiny = ctx.enter_context(
            tc.tile_pool(name="ps_tiny", bufs=2, space="PSUM"))
        ps_big = ctx.enter_context(
            tc.tile_pool(name="ps_big", bufs=2, space="PSUM"))
        ps_o = ctx.enter_context(
            tc.tile_pool(name="ps_o", bufs=1, space="PSUM"))

        # ---- constants / weights ------------------------------------
        identf = consts.tile([128, 128], f32, tag="identf")
        make_identity(nc, identf)
        ident16 = consts.tile([128, 128], f16, tag="ident16")
        nc.vector.tensor_copy(ident16, identf)
        ones128 = consts.tile([128, 1], f16, tag="ones128")
        nc.vector.memset(ones128, 1.0 / 128.0)
        ones128f = consts.tile([128, 1], f32, tag="ones128f")
        nc.vector.memset(ones128f, 1.0 / 128.0)
        ln2col = consts.tile([128, 1], f32, tag="ln2col")
        nc.vector.memset(ln2col, 0.6931471805599453)
        onesrow = consts.tile([1, DI], f16, tag="onesrow")
        nc.sync.dma_start(out=onesrow, in_=crow_d)

        def _load(name, dram_ap, shape, dt=f16):
            t = consts.tile(list(shape), dt, tag=name)
            nc.sync.dma_start(out=t, in_=dram_ap)
            return t

        wq_sb = [_load(f"wq{i}", wq_d[i], [128, DM]) for i in range(L)]
        wqf_sb = []
        for i in range(L):
            t = consts.tile([128, DM], f32, tag=f"wqf{i}")
            nc.vector.tensor_copy(t, wq_sb[i])
            wqf_sb.append(t)
        wk_sb = [_load(f"wk{i}", wk_d[i], [128, HE]) for i in range(L)]
        wv_sb = [_load(f"wv{i}", wv_d[i], [128, HE]) for i in range(L)]
        wfc_sb = [_load(f"wfc{i}", wfc_d[i], [128, DM]) for i in range(L)]
        w1_sb = [_load(f"w1{i}", w1_d[i], [128, DI]) for i in range(L)]
        w2_sb = [_load(f"w2{i}", w2_d[i], [128, DI]) for i in range(L)]
        ncq_sb = _load("ncq", ncq_d, [1, L * DM])
        ncqf_sb = consts.tile([1, L * DM], f32, tag="ncqf")
        nc.vector.tensor_copy(ncqf_sb, ncq_sb)
        b1r_sb = _load("b1r", b1r_d, [1, L * DI])
        b2_sb = _load("b2s", b2_d, [DM, L], f32)
        wl2a = _load("wl2a", wl2_d[0:DM], [128, DM])
        wl2b = _load("wl2b", wl2_d[DM:2 * DM], [128, DM])
        bl2_sb = _load("bl2", bl2_d, [128, 1], f32)
        if use_bq:
            bqr_sb = _load("bqr", bqr_d, [1, L * DM])
        if not use_mask:
            mb0_sb = _load("mb0", mb_d, [DM, HE], f32)

        def ap3(t, d1, d2, psrc=None):
            """view SBUF/PSUM 2D tile [128, d1*d2] as [128, d1, d2]"""
            src = psrc if psrc is not None else t
            return bass.AP(tensor=src.tensor, offset=src.offset,
                           ap=[src.ap[0], [d2, d1], [1, d2]])

        def bcast3(t, d1, d2):
            """[128, d1] -> [128, d1, d2] stride-0 broadcast"""
            return bass.AP(tensor=t.tensor, offset=t.offset,
                           ap=[t.ap[0], [1, d1], [0, d2]])

        # ---- per-layer emission --------------------------------------
        def rsqrt_neg(v):
            """[-rstd] via quake seed + 3 Newton iters, [128,16] f32."""
            yi = tinyp.tile([128, 16], i32, tag="yi")
            nc.vector.tensor_scalar(out=yi, in0=v.bitcast(i32), scalar1=1,
                                    scalar2=None, op0=OP.arith_shift_right)
            nc.vector.tensor_scalar(out=yi, in0=yi, scalar1=-1,
                                    scalar2=None, op0=OP.bitwise_xor)
            nc.vector.tensor_scalar(out=yi, in0=yi, scalar1=QK + 1,
                                    scalar2=None, op0=OP.add)
            y = yi.bitcast(f32)
            hv = tinyp.tile([128, 16], f32, tag="hv")
            nc.vector.tensor_scalar(out=hv, in0=v, scalar1=0.5, scalar2=None,
                                    op0=OP.mult)
            tq = tinyp.tile([128, 16], f32, tag="tq")
            for _ in range(3):
                nc.vector.tensor_mul(tq, y, y)
                nc.vector.tensor_mul(tq, tq, hv)
                nc.vector.scalar_tensor_tensor(out=y, in0=tq, scalar=1.5, in1=y,
                                               op0=OP.subtract, op1=OP.mult)
            return y

        def stats(xtok):
            """token-major xtok [128,2048] -> (mu, e2) [128,16] f32 SBUF."""
            mu = tinyp.tile([128, 16], f32, tag="mu")
            nc.vector.tensor_reduce(out=mu, in_=ap3(xtok, NT, 128), axis=AX,
                                    op=OP.add)
            nc.vector.tensor_scalar(out=mu, in0=mu, scalar1=1.0 / 128.0,
                                    scalar2=None, op0=OP.mult)
            sq = tmpp.tile([128, S], f16, tag="sqt")
            for c in range(4):
                nc.gpsimd.tensor_mul(sq[:, c * 512:(c + 1) * 512],
                                     xtok[:, c * 512:(c + 1) * 512],
                                     xtok[:, c * 512:(c + 1) * 512])
            e2 = tinyp.tile([128, 16], f32, tag="e2")
            nc.vector.tensor_reduce(out=e2, in_=ap3(sq, NT, 128), axis=AX,
                                    op=OP.add)
            return mu, e2

        def chain_nrstd(mu, e2):
            """(mu, e2) -> -rstd [128,16] f32 (e2 is a raw sum of squares)."""
            musq = tinyp.tile([128, 16], f32, tag="musq")
            nc.vector.tensor_mul(musq, mu, mu)
            vpe = tinyp.tile([128, 16], f32, tag="vpe")
            nc.vector.scalar_tensor_tensor(out=vpe, in0=e2,
                                           scalar=1.0 / 128.0,
                                           in1=musq, op0=OP.mult,
                                           op1=OP.subtract)
            nc.vector.tensor_scalar(out=vpe, in0=vpe, scalar1=float(EPS),
                                    scalar2=None, op0=OP.add)
            return rsqrt_neg(vpe)

        def tokmajor(x, eng=None):
            """feature-major [128,S] -> per-tile token-major via one DMA.
            out[p, t*128+f] = x[f, t*128+p]."""
            xt = workp.tile([128, NT, 128], f16, tag="xtok")
            (eng or nc.sync).dma_start_transpose(out=xt, in_=x)
            return xt.rearrange("p a b -> p (a b)")

        def emit_layer(i, xq, xkv, mb_sb, deep=False):
            # ---- token-major xq + LN1 stats + k projection ------------
            xqtok = tokmajor(xq)
            mu1, e21 = stats(xqtok)
            k_ps = ps_big.tile([128, 1024], f32, tag="big")
            for t in range(NT):
                nc.tensor.matmul(k_ps[:, t * HE:(t + 1) * HE],
                                 lhsT=xkv[:, t * 128:(t + 1) * 128],
                                 rhs=wk_sb[i])
            ty = ps_tiny.tile([128, 512], f32, tag="ty")
            nrstd = chain_nrstd(mu1, e21)
            rstd = tinyp.tile([128, 16], f32, tag="rstd")
            nc.vector.tensor_scalar(out=rstd, in0=nrstd, scalar1=-1.0,
                                    scalar2=None, op0=OP.mult)

            # ---- k scale (ACT, per token tile) ------------------------
            k_sb = workp.tile([128, NT * HE], f16, tag="ksb")
            for t in range(NT):
                nc.scalar.activation(k_sb[:, t * HE:(t + 1) * HE],
                                     k_ps[:, t * HE:(t + 1) * HE],
                                     AF.Identity, scale=rstd[:, t:t + 1])

            # ---- P accumulation + scores ------------------------------
            p_ps = ty[:, 0:HE]
            for t in range(NT):
                nc.tensor.matmul(p_ps,
                                 lhsT=xqtok[:, t * 128:(t + 1) * 128],
                                 rhs=k_sb[:, t * HE:(t + 1) * HE],
                                 start=(t == 0), stop=(t == NT - 1))
            p_sb = tinyp.tile([128, HE], f16, tag="psb")
            nc.vector.tensor_copy(p_sb, p_ps)
            s_ps = ty[0:1, HE:2 * HE]
            nc.tensor.matmul(s_ps, lhsT=ones128, rhs=p_sb)
            s_sb = tinyp.tile([1, HE], f16, tag="ssb")
            nc.vector.tensor_copy(s_sb, s_ps)
            sc_ps = ty[:, 2 * HE:3 * HE]
            nc.tensor.matmul(sc_ps, lhsT=wq_sb[i], rhs=p_sb,
                             start=True, stop=False)
            ncq_row = ncq_sb[:, i * DM:(i + 1) * DM]
            nc.tensor.matmul(sc_ps, lhsT=ncq_row, rhs=s_sb,
                             start=False, stop=(not use_bq))
            if use_bq:
                xsum = tinyp.tile([128, 1], f32, tag="xsum")
                nc.vector.tensor_reduce(out=xsum, in_=ap3(xkv, 4, 512),
                                        axis=mybir.AxisListType.XY, op=OP.add)
                xsum16 = tinyp.tile([128, 1], f16, tag="xsum16")
                nc.vector.tensor_copy(xsum16, xsum)
                ks_ps = ty[0:1, 3 * HE:4 * HE]
                nc.tensor.matmul(ks_ps, lhsT=xsum16, rhs=wk_sb[i])
                ks_sb = tinyp.tile([1, HE], f16, tag="kss")
                nc.vector.tensor_copy(ks_sb, ks_ps)
                nc.tensor.matmul(sc_ps, lhsT=bqr_sb[:, i * DM:(i + 1) * DM],
                                 rhs=ks_sb, start=False, stop=True)

            # ---- softmax -> block-diagonal attn (fp16) ----------------
            sm = tinyp.tile([128, HE], f32, tag="sm")
            nc.vector.tensor_tensor(out=sm, in0=sc_ps, in1=mb_sb, op=OP.add)
            negmx = tinyp.tile([128, 1], f32, tag="negmx")
            nc.vector.tensor_reduce(out=negmx, in_=sm, axis=AX,
                                    op=OP.max, negate=True)
            sm2 = tinyp.tile([128, HE], f32, tag="sm2")
            nc.vector.tensor_scalar(out=sm2, in0=sm, scalar1=negmx,
                                    scalar2=None, op0=OP.add)
            es = tinyp.tile([128, HE], f32, tag="es")
            nc.scalar.activation(es, sm2, AF.Exp)
            ssum = tinyp.tile([128, H], f32, tag="ssum")
            nc.vector.tensor_reduce(out=ssum, in_=ap3(es, H, DKP), axis=AX,
                                    op=OP.add)
            nc.vector.tensor_scalar(out=ssum, in0=ssum, scalar1=1e-30,
                                    scalar2=None, op0=OP.add)
            rs = tinyp.tile([128, H], f32, tag="rs")
            nc.vector.reciprocal(rs, ssum)
            bda = tinyp.tile([128, HE], f16, tag="bda")
            nc.vector.tensor_tensor(out=ap3(bda, H, DKP), in0=ap3(es, H, DKP),
                                    in1=bcast3(rs, H, DKP), op=OP.mult)

            # ---- C = bda^T wfcP ---------------------------------------
            c_ps = ty[0:HE, 4 * HE:4 * HE + 128]
            nc.tensor.matmul(c_ps, lhsT=bda, rhs=wfc_sb[i])
            c_sb = tinyp.tile([HE, 128], f16, tag="csb")
            nc.vector.tensor_copy(c_sb, c_ps)

            # ---- v projection + attn out + residual -------------------
            vT = workp.tile([HE, S], f16, tag="vt")
            for c in range(4):
                v_ps = ps_big.tile([HE, 512], f32, tag="big")
                nc.tensor.matmul(v_ps, lhsT=wv_sb[i],
                                 rhs=xkv[:, c * 512:(c + 1) * 512])
                nc.scalar.copy(out=vT[:, c * 512:(c + 1) * 512], in_=v_ps)
            out1 = workp.tile([128, S], f16, tag="o1")
            for c in range(4):
                cs = slice(c * 512, (c + 1) * 512)
                ao_ps = ps_big.tile([128, 512], f32, tag="big")
                nc.tensor.matmul(ao_ps, lhsT=c_sb, rhs=vT[:, cs])
                nc.vector.tensor_tensor(out=out1[:, cs], in0=ao_ps,
                                        in1=xq[:, cs], op=OP.add)

            # ---- LN2 stats + rows -------------------------------------
            o1tok = tokmajor(out1, eng=nc.scalar)
            mu2, e22 = stats(o1tok)
            nrstd2 = chain_nrstd(mu2, e22)
            r2h = tinyp.tile([128, 16], f16, tag="r2h")
            nc.vector.tensor_scalar(out=r2h, in0=nrstd2, scalar1=-1.0,
                                    scalar2=None, op0=OP.mult)
            nm2h = tinyp.tile([128, 16], f16, tag="nm2h")
            nc.vector.tensor_tensor(out=nm2h, in0=mu2, in1=nrstd2, op=OP.mult)
            tr_ps = ty[0:16, 384:512].bitcast(f16)
            nc.tensor.transpose(tr_ps[:, 0:128], r2h, ident16)
            nc.tensor.transpose(tr_ps[:, 128:256], nm2h, ident16)
            rows = rowp.tile([16, 256], f16, tag="rows")
            nc.vector.tensor_copy(rows, tr_ps)
            rowrow = rowp.tile([1, 2 * S], f16, tag="rr")
            r2row = rowrow[:, 0:S]
            nmrow = rowrow[:, S:2 * S]
            nc.scalar.dma_start(out=r2row, in_=rows[:, 0:128])
            nc.scalar.dma_start(out=nmrow, in_=rows[:, 128:256])

            # ---- FFN per 1024-token chunk -----------------------------
            out2 = statep.tile([128, S], f16, tag="state")
            for c2 in range(2):
                cs2 = slice(c2 * 1024, (c2 + 1) * 1024)
                n2c = tmpp.tile([128, 1024], f16, tag="n2c")
                for cc in range(2):
                    c = 2 * c2 + cc
                    s5 = slice(c * 512, (c + 1) * 512)
                    l5 = slice(cc * 512, (cc + 1) * 512)
                    bc_ps = ps_big.tile([128, 1024], f32, tag="big")
                    nc.tensor.matmul(bc_ps[:, 0:512],
                                     lhsT=onesrow[:, 0:128], rhs=r2row[:, s5])
                    nc.tensor.matmul(bc_ps[:, 512:1024],
                                     lhsT=onesrow[:, 0:128], rhs=nmrow[:, s5])
                    tmp = tmpp.tile([128, 512], f16, tag="tmp")
                    nc.vector.tensor_tensor(
                        out=tmp, in0=out1[:, s5],
                        in1=bc_ps[:, 0:512], op=OP.mult)
                    nc.vector.tensor_tensor(out=n2c[:, l5], in0=tmp,
                                            in1=bc_ps[:, 512:1024], op=OP.add)
                o_ps = ps_o.tile([128, 1024], f32, tag="ops")
                for j in range(4):
                    h_ps = ps_big.tile([128, 1024], f32, tag="big")
                    w1j = w1_sb[i][:, j * 128:(j + 1) * 128]
                    b1row = b1r_sb[:, (i * 4 + j) * 128:(i * 4 + j + 1) * 128]
                    nc.tensor.matmul(h_ps[:, 0:512], lhsT=w1j,
                                     rhs=n2c[:, 0:512], start=True, stop=False)
                    nc.tensor.matmul(h_ps[:, 0:512], lhsT=b1row,
                                     rhs=onesrow[:, 0:512],
                                     start=False, stop=True)
                    nc.tensor.matmul(h_ps[:, 512:1024], lhsT=w1j,
                                     rhs=n2c[:, 512:1024],
                                     start=True, stop=False)
                    nc.tensor.matmul(h_ps[:, 512:1024], lhsT=b1row,
                                     rhs=onesrow[:, 0:512],
                                     start=False, stop=True)
                    hsb = tmpp.tile([128, 1024], f16, tag="hsb")
                    if deep:
                        u = tmpp.tile([128, 1024], f32, tag="u32")
                        nc.scalar.activation(u, h_ps, AF.Exp)
                        w = tmpp.tile([128, 1024], f32, tag="w")
                        nc.scalar.activation(w, u, AF.Square, bias=1.0)
                        q_ = tmpp.tile([128, 1024], f32, tag="q_")
                        nc.scalar.activation(q_, w, AF.Ln, bias=1.0)
                        r_ = tmpp.tile([128, 1024], f32, tag="r_")
                        nc.scalar.activation(r_, q_, AF.Exp, scale=-1.0,
                                             bias=ln2col)
                        t_ = tmpp.tile([128, 1024], f16, tag="t_")
                        nc.vector.tensor_scalar(out=t_, in0=r_, scalar1=-1.0,
                                                scalar2=1.0, op0=OP.mult,
                                                op1=OP.add)
                        nc.vector.scalar_tensor_tensor(
                            out=hsb, in0=h_ps, scalar=0.0, in1=t_,
                            op0=OP.add, op1=OP.mult)
                    else:
                        u = tmpp.tile([128, 1024], f16, tag="u")
                        nc.scalar.activation(u, h_ps, AF.Exp)
                        w = tmpp.tile([128, 1024], f32, tag="w")
                        nc.scalar.activation(w, u, AF.Square, bias=1.0)
                        _emit_misht(nc, hsb, h_ps, w)
                    nc.tensor.matmul(o_ps[:, 0:512],
                                     lhsT=w2_sb[i][:, j * 128:(j + 1) * 128],
                                     rhs=hsb[:, 0:512],
                                     start=(j == 0), stop=(j == 3))
                    nc.tensor.matmul(o_ps[:, 512:1024],
                                     lhsT=w2_sb[i][:, j * 128:(j + 1) * 128],
                                     rhs=hsb[:, 512:1024],
                                     start=(j == 0), stop=(j == 3))
                nc.vector.scalar_tensor_tensor(
                    out=out2[:, cs2], in0=o_ps, scalar=b2_sb[:, i:i + 1],
                    in1=out1[:, cs2], op0=OP.add, op1=OP.add)
            return out2

        # ---- main item loop ------------------------------------------
        with tc.For_i(0, n_items, 1, staggered_reset=True) as it:
            x1 = bigp.tile([128, S], f16, tag="x1")
            x2 = bigp.tile([128, S], f16, tag="x2")
            xi = xin[bass.ds(it, 1)].squeeze(0)
            nc.sync.dma_start(out=x1, in_=xi[0:128])
            nc.sync.dma_start(out=x2, in_=xi[128:256])
            if use_mask:
                mb_sb = tinyp.tile([DM, HE], f32, tag="mb")
                nc.sync.dma_start(out=mb_sb,
                                  in_=mb_d[bass.ds(it, 1)].squeeze(0))
            else:
                mb_sb = mb0_sb

            sA, sB = x1, x2
            for i in range(L):
                sA = emit_layer(i, sA, sA, mb_sb)
                sB = emit_layer(i, sB, sB, mb_sb)
            nc.scalar.dma_start(out=m1_o[bass.ds(it, 1)].squeeze(0), in_=sA)
            nc.scalar.dma_start(out=m2_o[bass.ds(it, 1)].squeeze(0), in_=sB)

            eA, eB = x2, x1
            for i in range(L):
                kvA = x1 if i == 0 else eA
                kvB = x2 if i == 0 else eB
                eA = emit_layer(i, eA, kvA, mb_sb, deep=True)
                eB = emit_layer(i, eB, kvB, mb_sb, deep=True)

            enc = statep.tile([128, S], f16, tag="state")
            for c in range(4):
                cs = slice(c * 512, (c + 1) * 512)
                en_ps = ps_big.tile([128, 512], f32, tag="big")
                nc.tensor.matmul(en_ps, lhsT=wl2a, rhs=eA[:, cs],
                                 start=True, stop=False)
                nc.tensor.matmul(en_ps, lhsT=wl2b, rhs=eB[:, cs],
                                 start=False, stop=True)
                nc.vector.tensor_scalar(out=enc[:, cs], in0=en_ps,
                                        scalar1=bl2_sb, scalar2=None,
                                        op0=OP.add)
            for i in range(L):
                enc = emit_layer(i, enc, enc, mb_sb, deep=True)
            nc.scalar.dma_start(out=e_o[bass.ds(it, 1)].squeeze(0), in_=enc)

    nc.compile()
    return nc


_CACHE = {}


def _get_built(n_items, use_bq, use_mask):
    key = (n_items, use_bq, use_mask)
    if key not in _CACHE:
        _CACHE[key] = build(n_items, use_bq, use_mask)
    return _CACHE[key]


def _in_maps(f, src16, n_items, n_cores, use_bq, use_mask):
    base = {
        'wq': f['wq'], 'ncq': f['ncq'].reshape(1, L * DM), 'wk': f['wk'],
        'wv': f['wv'], 'wfc': f['wfc'], 'w1': f['w1'], 'b1r': f['b1r'],
        'w2r': f['w2r'], 'b2s': f['b2s'], 'wl2': f['wl2'], 'bl2': f['bl2'],
        'crow': np.ones((1, DI), np.float16),
    }
    if use_bq:
        base['bqr'] = f['bqr'].reshape(1, L * DM)
    if not use_mask:
        base['mb0'] = f['mb0']
    maps = []
    for c in range(n_cores):
        m = dict(base)
        m['xin'] = src16[c * n_items:(c + 1) * n_items]
        if use_mask:
            m['mb'] = np.ascontiguousarray(
                f['mb'][c * n_items:(c + 1) * n_items])
        maps.append(m)
    return maps


def run(inputs, trace=False):
    from concourse import bass_utils
    from concourse.bass_utils import run_bass_kernel_spmd
    if trace:
        import ntff_shim
        ntff_shim.install()
        bass_utils.upload_artifacts = lambda tmpdir: tmpdir
    f = fold_weights(inputs)
    use_bq = not f['bq_trivial']
    use_mask = not f['mask_trivial']
    src = np.asarray(inputs['src_seq'], np.float32)
    nb = src.shape[0]
    n_cores = N_CORES if nb % N_CORES == 0 else 1
    n_items = nb // n_cores
    # host: feature-major fp16
    src16 = np.ascontiguousarray(src.transpose(0, 2, 1).astype(np.float16))
    nc = _get_built(n_items, use_bq, use_mask)
    maps = _in_maps(f, src16, n_items, n_cores, use_bq, use_mask)
    res = run_bass_kernel_spmd(nc, maps, core_ids=list(range(n_cores)),
                               trace=trace, trace_cores=[0] if trace else None)

    def gather(name):
        parts = [res.results[c][name] for c in range(n_cores)]
        full = np.concatenate(parts, 0)          # [B, 128, S] f16
        return np.ascontiguousarray(
            full.transpose(0, 2, 1).astype(np.float32))
    return (gather('eo'), gather('m1o'), gather('m2o')), res


def kernel(**inputs):
    (enc, m1, m2), _ = run(inputs, trace=False)
    return (enc, m1, m2)
